# revision 9
# baseline (speedup 1.0000x reference)
"""Multi-head causal attention with RoPE on 8 Trainium2 NeuronCores.

Sharding: data-parallel over batch (2 groups of 4 cores) x tensor-parallel
over heads (4 heads / 512 cols of Wq/Wk/Wv per core, 512 rows of Wo).
Each core computes its head-group's Q/K/V projections in transposed layout
([head_dim, seq] -- so no on-device transposes are ever needed), applies
RoPE, runs causal softmax attention (scores kept transposed [tk, tq];
row sums via a ones-vector matmul), and emits its partial output
projection.  The host sums the 4 partials per batch element.

Self-contained: shapes/sharding hardcoded for
  q_input/kv_input [2, 2048, 2048], 16 heads x 128 head_dim.
"""

import math

import numpy as np
import ml_dtypes

B, T, D, H = 2, 2048, 2048, 16
HD = 128          # head dim
HALF = HD // 2    # rope half
P = 128           # partitions
CHUNK = 512       # tq / free-dim chunk
NCORES = 8
GROUPS = 4        # head-groups (tensor-parallel degree per batch)
HPG = H // GROUPS # heads per group
GD = HPG * HD     # group width (512)
DT = D // P       # d-tiles (16)
TCH = T // CHUNK  # seq chunks (4)
TKT = T // P      # tk tiles (16)
CPT = CHUNK // P  # tk tiles per chunk (4)

TRACE = False       # set True before calling kernel() to capture an NTFF trace
LAST_RESULT = None  # BassKernelResults of the last kernel() call
DEBUG_DUMPS = False # add QT/KT/V/AT debug outputs to the program

_cache = {}


def _build_program(actions, npat):
    """Build the per-core Bass program.

    actions: {(c, t): "full" | pattern_index} for every (tq-chunk, tk-tile)
    score block that has at least one unmasked element.
    """
    from contextlib import ExitStack

    import concourse.mybir as mybir
    import concourse.tile as tile
    from concourse import bacc
    from concourse.bass import ts

    fp32 = mybir.dt.float32
    bf16 = mybir.dt.bfloat16
    Copy = mybir.ActivationFunctionType.Copy
    Exp = mybir.ActivationFunctionType.Exp
    SCALE = 1.0 / math.sqrt(HD)

    nc = bacc.Bacc(
        "TRN2",
        target_bir_lowering=False,
        debug=False,
        enable_asserts=False,
        num_devices=NCORES,
    )

    xqT = nc.dram_tensor("xqT", [D, T], bf16, kind="ExternalInput").ap()
    xkvT = nc.dram_tensor("xkvT", [D, T], bf16, kind="ExternalInput").ap()
    wq = nc.dram_tensor("wq", [D, GD], bf16, kind="ExternalInput").ap()
    wk = nc.dram_tensor("wk", [D, GD], bf16, kind="ExternalInput").ap()
    wv = nc.dram_tensor("wv", [D, GD], bf16, kind="ExternalInput").ap()
    wo = nc.dram_tensor("wo", [GD, D], bf16, kind="ExternalInput").ap()
    # RoPE in head-dim-interleaved space (host permutes Wq/Wk columns so the
    # rope pair (j, j+64) lands on adjacent partitions (2j, 2j+1); scores are
    # invariant to a common Q/K head-dim permutation):
    #   rope'(x) = x * cs2 + swap_adjacent_pairs(x) * ss2
    # cs2[2j] = cs2[2j+1] = cos_j ; ss2[2j] = -sin_j, ss2[2j+1] = +sin_j
    cs2 = nc.dram_tensor("cs2", [P, T], bf16, kind="ExternalInput").ap()
    ss2 = nc.dram_tensor("ss2", [P, T], bf16, kind="ExternalInput").ap()
    pat = nc.dram_tensor("pat", [npat, P, CHUNK], bf16, kind="ExternalInput").ap()
    out = nc.dram_tensor("out", [T, D], bf16, kind="ExternalOutput").ap()

    with ExitStack() as ctx:
        tc = ctx.enter_context(tile.TileContext(nc))
        const_pool = ctx.enter_context(tc.tile_pool(name="const", bufs=1))
        xpool = ctx.enter_context(tc.tile_pool(name="xchunk", bufs=2))
        rope_pool = ctx.enter_context(tc.tile_pool(name="rope", bufs=3))
        exp_pool = ctx.enter_context(tc.tile_pool(name="exp", bufs=3))
        osb_pool = ctx.enter_context(tc.tile_pool(name="osb", bufs=3))
        lb_pool = ctx.enter_context(tc.tile_pool(name="lb", bufs=2))
        mm_psum = ctx.enter_context(tc.tile_pool(name="mmps", bufs=2, space="PSUM"))
        s_psum = ctx.enter_context(tc.tile_pool(name="sps", bufs=2, space="PSUM"))
        o_psum = ctx.enter_context(tc.tile_pool(name="ops", bufs=2, space="PSUM"))
        l_psum = ctx.enter_context(tc.tile_pool(name="lps", bufs=2, space="PSUM"))

        # persistent SBUF tensors
        wq_sb = const_pool.tile([P, DT, GD], bf16, tag="wq")
        wk_sb = const_pool.tile([P, DT, GD], bf16, tag="wk")
        wv_sb = const_pool.tile([P, DT, GD], bf16, tag="wv")
        wo_sb = const_pool.tile([P, HPG, D], bf16, tag="wo")
        cs2_sb = const_pool.tile([P, T], bf16, tag="cs2")
        ss2_sb = const_pool.tile([P, T], bf16, tag="ss2")
        pat_sb = const_pool.tile([P, npat, CHUNK], bf16, tag="pat")
        ones_sb = const_pool.tile([P, 1], bf16, tag="ones")
        QT = const_pool.tile([P, HPG, T], bf16, tag="QT")
        KT = const_pool.tile([P, HPG, T], bf16, tag="KT")
        V = const_pool.tile([P, TKT, GD], bf16, tag="V")
        AT = const_pool.tile([P, HPG, T], bf16, tag="AT")

        # DMA issue order matters for time-to-first-matmul: the K/V loop
        # only needs wk/wv/cs2/ss2 + its first x chunk.  wq is issued just
        # before the Q loop, pat/wo during later stages.
        nc.sync.dma_start(wk_sb[:], wk.rearrange("(dt p) n -> p dt n", p=P))
        nc.sync.dma_start(cs2_sb[:], cs2)
        nc.sync.dma_start(ss2_sb[:], ss2)
        nc.sync.dma_start(wv_sb[:], wv.rearrange("(dt p) n -> p dt n", p=P))
        nc.vector.memset(ones_sb[:], 1.0)

        SHUF_MASK = [i + 1 - 2 * (i % 2) for i in range(32)]  # [1,0,3,2,...]

        def rope_evict(ps, h, c, dest):
            # ps: PSUM [P, CHUNK] fp32, partitions = interleaved head_dim
            raw = rope_pool.tile([P, CHUNK], bf16, tag="raw")
            nc.scalar.activation(raw[:], ps[:], Copy)
            rsw = rope_pool.tile([P, CHUNK], bf16, tag="rsw")
            nc.vector.stream_shuffle(rsw[:], raw[:], SHUF_MASK)
            t1 = rope_pool.tile([P, CHUNK], bf16, tag="t1")
            nc.vector.tensor_mul(t1[:], raw[:], cs2_sb[:, ts(c, CHUNK)])
            t2 = rope_pool.tile([P, CHUNK], bf16, tag="t2")
            nc.vector.tensor_mul(t2[:], rsw[:], ss2_sb[:, ts(c, CHUNK)])
            nc.vector.tensor_add(dest[:, h, ts(c, CHUNK)], t1[:], t2[:])

        # K^T / V from kv_input, Q^T from q_input
        for c in range(TCH):
            xk = xpool.tile([P, DT, CHUNK], bf16, tag="xc")
            nc.sync.dma_start(
                xk[:], xkvT.rearrange("(dt p) t -> p dt t", p=P)[:, :, ts(c, CHUNK)]
            )
            for h in range(HPG):
                ps = mm_psum.tile([P, CHUNK], fp32, tag="mm")
                for d in range(DT):
                    nc.tensor.matmul(
                        ps[:], wk_sb[:, d, ts(h, HD)], xk[:, d, :],
                        start=(d == 0), stop=(d == DT - 1),
                    )
                rope_evict(ps, h, c, KT)
            for s in range(CPT):
                ps = mm_psum.tile([P, GD], fp32, tag="mm")
                for d in range(DT):
                    nc.tensor.matmul(
                        ps[:], xk[:, d, ts(s, P)], wv_sb[:, d, :],
                        start=(d == 0), stop=(d == DT - 1),
                    )
                nc.scalar.activation(V[:, c * CPT + s, :], ps[:], Copy)
        nc.sync.dma_start(wq_sb[:], wq.rearrange("(dt p) n -> p dt n", p=P))
        nc.sync.dma_start(pat_sb[:], pat.rearrange("j p n -> p j n"))
        for c in range(TCH):
            xq = xpool.tile([P, DT, CHUNK], bf16, tag="xc")
            nc.sync.dma_start(
                xq[:], xqT.rearrange("(dt p) t -> p dt t", p=P)[:, :, ts(c, CHUNK)]
            )
            for h in range(HPG):
                ps = mm_psum.tile([P, CHUNK], fp32, tag="mm")
                for d in range(DT):
                    nc.tensor.matmul(
                        ps[:], wq_sb[:, d, ts(h, HD)], xq[:, d, :],
                        start=(d == 0), stop=(d == DT - 1),
                    )
                rope_evict(ps, h, c, QT)
        nc.sync.dma_start(wo_sb[:], wo.rearrange("(h p) n -> p h n", p=P))

        # attention: scores kept transposed [tk, tq]; O^T accumulated in PSUM
        for c in range(TCH):
            for h in range(HPG):
                opst = o_psum.tile([P, CHUNK], fp32, tag="o")
                lpst = l_psum.tile([1, CHUNK], fp32, tag="l")
                tlist = [t for t in range(TKT) if (c, t) in actions]
                for i, t in enumerate(tlist):
                    spst = s_psum.tile([P, CHUNK], fp32, tag="s")
                    nc.tensor.matmul(
                        spst[:], KT[:, h, ts(t, P)], QT[:, h, ts(c, CHUNK)],
                        start=True, stop=True,
                    )
                    es = exp_pool.tile([P, CHUNK], bf16, tag="es")
                    nc.scalar.activation(es[:], spst[:], Exp, scale=SCALE)
                    a = actions[(c, t)]
                    if a != "full":
                        nc.vector.tensor_mul(es[:], es[:], pat_sb[:, a, :])
                    first, last = (i == 0), (i == len(tlist) - 1)
                    nc.tensor.matmul(
                        lpst[:], ones_sb[:], es[:], start=first, stop=last
                    )
                    nc.tensor.matmul(
                        opst[:], V[:, t, ts(h, HD)], es[:], start=first, stop=last
                    )
                rec = lb_pool.tile([1, CHUNK], fp32, tag="rec")
                nc.vector.reciprocal(rec[:], lpst[:])
                # broadcast 1/l across partitions on the (idle) GpSimd engine
                lbs = lb_pool.tile([P, CHUNK], fp32, tag="lbs")
                nc.gpsimd.partition_broadcast(lbs[:], rec[:])
                nc.vector.tensor_mul(AT[:, h, ts(c, CHUNK)], opst[:], lbs[:])

        if DEBUG_DUMPS:
            for nm, sb in [("dQT", QT), ("dKT", KT), ("dV", V), ("dAT", AT)]:
                dt_ = nc.dram_tensor(nm, list(sb.shape), bf16, kind="ExternalOutput").ap()
                nc.sync.dma_start(dt_[:], sb[:])

        # partial output projection: out[tq, :] = sum_h attn_h^T.T @ Wo_h
        OCH = min(CHUNK, D)
        for oc in range(D // OCH):
            for m in range(TKT):
                ps = mm_psum.tile([P, OCH], fp32, tag="mm")
                for h in range(HPG):
                    nc.tensor.matmul(
                        ps[:], AT[:, h, ts(m, P)], wo_sb[:, h, ts(oc, OCH)],
                        start=(h == 0), stop=(h == HPG - 1),
                    )
                ob = osb_pool.tile([P, OCH], bf16, tag="ob")
                nc.scalar.activation(ob[:], ps[:], Copy)
                nc.sync.dma_start(out[ts(m, P), ts(oc, OCH)], ob[:])

    nc.compile()
    return nc


def _interleave_heads(W):
    """Permute each 128-wide head block of columns: new[2j]=old[j], new[2j+1]=old[64+j]."""
    d, gd = W.shape
    return np.ascontiguousarray(
        W.reshape(d, gd // HD, 2, HALF).transpose(0, 1, 3, 2).reshape(d, gd)
    )


def _rope_tables(cos, sin):
    """cs2[2j]=cs2[2j+1]=cos_j ; ss2[2j]=-sin_j, ss2[2j+1]=+sin_j  (both [128, T])."""
    bf = ml_dtypes.bfloat16
    cosT = np.ascontiguousarray(cos.T)  # [HALF, T]
    sinT = np.ascontiguousarray(sin.T)
    cs2 = np.repeat(cosT, 2, axis=0).astype(bf)
    ss2 = np.stack([-sinT, sinT], axis=1).reshape(HD, -1).astype(bf)
    return cs2, ss2


def _mask_actions(mask):
    """Classify every [CHUNK tq x P tk] score block of the mask.

    Returns ({(c, t): "full" | pattern_idx}, patterns [npat, P, CHUNK] bf16).
    Blocks with no unmasked element are omitted (skipped entirely).
    Patterns are stored transposed ([tk, tq]) to match the score layout.
    """
    m = np.asarray(mask).reshape(T, T).astype(bool)
    actions = {}
    pats = []
    pat_keys = {}
    for c in range(TCH):
        for t in range(TKT):
            blk = m[c * CHUNK : (c + 1) * CHUNK, t * P : (t + 1) * P]
            if not blk.any():
                continue
            if blk.all():
                actions[(c, t)] = "full"
                continue
            bt = np.ascontiguousarray(blk.T)
            key = bt.tobytes()
            if key not in pat_keys:
                pat_keys[key] = len(pats)
                pats.append(bt.astype(ml_dtypes.bfloat16))
            actions[(c, t)] = pat_keys[key]
    if not pats:
        pats.append(np.zeros((P, CHUNK), ml_dtypes.bfloat16))
    return actions, np.ascontiguousarray(np.stack(pats))


def kernel(**inputs):
    global LAST_RESULT
    q_input = np.asarray(inputs["q_input"], dtype=np.float32)
    kv_input = np.asarray(inputs["kv_input"], dtype=np.float32)
    cos = np.asarray(inputs["cos"], dtype=np.float32)
    sin = np.asarray(inputs["sin"], dtype=np.float32)
    Wq = np.asarray(inputs["Wq"], dtype=np.float32)
    Wk = np.asarray(inputs["Wk"], dtype=np.float32)
    Wv = np.asarray(inputs["Wv"], dtype=np.float32)
    Wo = np.asarray(inputs["Wo"], dtype=np.float32)

    actions, pats = _mask_actions(inputs["mask"])
    key = (tuple(sorted(actions.items())), pats.shape[0])
    if key not in _cache:
        _cache[key] = _build_program(actions, int(pats.shape[0]))
    nc = _cache[key]

    bf = ml_dtypes.bfloat16
    cs2, ss2 = _rope_tables(cos, sin)
    xq = [np.ascontiguousarray(q_input[b].T).astype(bf) for b in range(B)]
    xkv = [np.ascontiguousarray(kv_input[b].T).astype(bf) for b in range(B)]
    wq_g = [_interleave_heads(Wq[:, g * GD : (g + 1) * GD]).astype(bf) for g in range(GROUPS)]
    wk_g = [_interleave_heads(Wk[:, g * GD : (g + 1) * GD]).astype(bf) for g in range(GROUPS)]
    wv_g = [np.ascontiguousarray(Wv[:, g * GD : (g + 1) * GD]).astype(bf) for g in range(GROUPS)]
    wo_g = [np.ascontiguousarray(Wo[g * GD : (g + 1) * GD, :]).astype(bf) for g in range(GROUPS)]

    in_maps = []
    for core in range(NCORES):
        b, g = divmod(core, GROUPS)
        in_maps.append({
            "xqT": xq[b],
            "xkvT": xkv[b],
            "wq": wq_g[g],
            "wk": wk_g[g],
            "wv": wv_g[g],
            "wo": wo_g[g],
            "cs2": cs2,
            "ss2": ss2,
            "pat": pats,
        })

    from concourse import bass_utils

    res = bass_utils.run_bass_kernel_spmd(
        nc, in_maps, core_ids=list(range(NCORES)), trace=TRACE
    )
    LAST_RESULT = res
    outs = [np.asarray(r["out"], dtype=np.float32) for r in res.results]
    full = np.stack(
        [sum(outs[b * GROUPS + g] for g in range(GROUPS)) for b in range(B)]
    )
    return np.ascontiguousarray(full.astype(np.float32))

